# revision 6
# baseline (speedup 1.0000x reference)
"""Multi-head causal attention (B=4, S=2048, D=1024, H=16) on 8 TRN2 NeuronCores.

Sharding: core c handles batch b = c//2 and head-group hg = c%2 (8 heads each).
Each core computes Q/K/V projections for its (batch, head-group), causal
attention, and a partial output projection over its 512 head-dims.  The host
sums the two partials per batch and adds b_o.  No collectives.

Device-side layout choices:
  - x is passed transposed (xT [D, S]) so projection matmuls contract over
    partitions directly.
  - Q and K are produced transposed (QT/KT [dq, S]); scores are computed
    transposed (S^T [kpos, q]) which makes the softmax denominator a matmul
    with a ones-column (no partition reductions anywhere).
  - No max-subtraction in softmax: scaled scores are ~N(0,1), exp is safe.
  - P (=exp(scores)) and V are bf16 for the P@V matmul; everything else is
    float32r (full-rate fp32 on the PE).
"""

import sys
import os

sys.path.insert(0, "/opt/trn_rl_repo")

import numpy as np

import concourse.bacc as bacc
import concourse.mybir as mybir
import concourse.tile as tile
from concourse.bass_utils import run_bass_kernel_spmd

B, S, D, H = 4, 2048, 1024, 16
DK = D // H          # 64
HH = H // 2          # 8 heads per core
HD = HH * DK         # 512 head-dims per core
N_CORES = 8

F32 = mybir.dt.float32
F32R = mybir.dt.float32r
BF16 = mybir.dt.bfloat16

SCALE = 1.0 / np.sqrt(DK)


def build_nc(s=S, interleave_pairs=True):
    """Build the per-core SPMD program.  `s` is the sequence length (tunable
    for small-scale simulation; must be a multiple of 512)."""
    assert s % 512 == 0
    n_qb = s // 512          # 512-wide q blocks
    n_t128 = s // 128        # 128-wide token tiles
    n_dt = D // 128          # din tiles (8)

    nc = bacc.Bacc("TRN2", target_bir_lowering=False, debug=False,
                   num_devices=N_CORES)

    xT = nc.dram_tensor("xT", [D, s], F32R, kind="ExternalInput")
    wqT = nc.dram_tensor("wqT", [D, HD], F32R, kind="ExternalInput")
    wkT = nc.dram_tensor("wkT", [D, HD], F32R, kind="ExternalInput")
    wvT = nc.dram_tensor("wvT", [D, HD], F32R, kind="ExternalInput")
    woT = nc.dram_tensor("woT", [HD, D], F32R, kind="ExternalInput")
    out = nc.dram_tensor("out", [s, D], F32, kind="ExternalOutput")

    with tile.TileContext(nc) as tc:
        with tc.tile_pool(name="persist", bufs=1) as persist:
            # Persistent SBUF arrays (live for the whole kernel).
            qt_sb = [persist.tile([128, s], F32R, tag=f"qt{d}", name=f"qt{d}") for d in range(HD // 128)]
            kt_sb = [persist.tile([128, s], F32R, tag=f"kt{d}", name=f"kt{d}") for d in range(HD // 128)]
            # V tiles hold [t, head, 2*dk]: cols 0-63 are V, cols 64-127 are 1.0.
            # As the AV stationary this makes the matmul emit U^T on psum rows
            # 0-63 and the softmax denominator broadcast over rows 64-127.
            v_sb = [persist.tile([128, HH, 2 * DK], BF16, tag=f"v{t}", name=f"v{t}") for t in range(n_t128)]
            wo_sb = [persist.tile([128, D], F32R, tag=f"wo{d}", name=f"wo{d}") for d in range(HD // 128)]

            for d in range(HD // 128):
                nc.sync.dma_start(out=wo_sb[d][:], in_=woT[d * 128:(d + 1) * 128, :])

            # ---------------- Phase 1: projections ----------------
            with tc.tile_pool(name="xt", bufs=1) as xt_pool, \
                 tc.tile_pool(name="wload", bufs=10) as wload, \
                 tc.tile_pool(name="ppsum", bufs=4, space="PSUM") as ppsum:

                xt = []
                for i in range(n_dt):
                    t = xt_pool.tile([128, s], F32R, tag=f"x{i}", name=f"x{i}")
                    nc.sync.dma_start(out=t[:], in_=xT[i * 128:(i + 1) * 128, :])
                    xt.append(t)

                # QT / KT: [dq, t] = sum_din wT[din, dq]^T xT[din, t]
                for (wdram, dst) in ((wqT, qt_sb), (wkT, kt_sb)):
                    w_tiles = []
                    for i in range(n_dt):
                        wt = wload.tile([128, HD], F32R, tag="w", name="w")
                        nc.sync.dma_start(out=wt[:], in_=wdram[i * 128:(i + 1) * 128, :])
                        w_tiles.append(wt)
                    for dq in range(HD // 128):
                        for tb in range(n_qb):
                            ps = ppsum.tile([128, 512], F32, tag="pp", name="pp")
                            for i in range(n_dt):
                                nc.tensor.matmul(
                                    ps[:],
                                    lhsT=w_tiles[i][:, dq * 128:(dq + 1) * 128],
                                    rhs=xt[i][:, tb * 512:(tb + 1) * 512],
                                    start=(i == 0), stop=(i == n_dt - 1),
                                )
                            nc.vector.tensor_copy(
                                out=dst[dq][:, tb * 512:(tb + 1) * 512], in_=ps[:])

                # V: [t, dv] = sum_din xT[din, t]^T wvT[din, dv]
                wv_tiles = []
                for i in range(n_dt):
                    wt = wload.tile([128, HD], F32R, tag="w", name="w")
                    nc.sync.dma_start(out=wt[:], in_=wvT[i * 128:(i + 1) * 128, :])
                    wv_tiles.append(wt)
                for tb in range(n_t128):
                    ps = ppsum.tile([128, 512], F32, tag="pp", name="pp")
                    for i in range(n_dt):
                        nc.tensor.matmul(
                            ps[:],
                            lhsT=xt[i][:, tb * 128:(tb + 1) * 128],
                            rhs=wv_tiles[i][:],
                            start=(i == 0), stop=(i == n_dt - 1),
                        )
                    # ones columns for the denominator, then V data (cast bf16)
                    nc.vector.memset(v_sb[tb][:, :, DK:2 * DK], 1.0)
                    nc.vector.tensor_copy(
                        out=v_sb[tb][:, :, 0:DK],
                        in_=ps[:].rearrange("p (h k) -> p h k", h=HH))

            # ---------------- Phase 2: attention + out-projection ----------------
            with tc.tile_pool(name="pT", bufs=40) as pT_pool, \
                 tc.tile_pool(name="aoT", bufs=6) as aoT_pool, \
                 tc.tile_pool(name="rb", bufs=4) as rb_pool, \
                 tc.tile_pool(name="outsb", bufs=3) as out_pool, \
                 tc.tile_pool(name="spsum", bufs=3, space="PSUM") as spsum, \
                 tc.tile_pool(name="upsum", bufs=3, space="PSUM") as upsum, \
                 tc.tile_pool(name="opsum", bufs=2, space="PSUM") as opsum:

                for qb in range(n_qb):
                    nkt = 4 * qb + 4
                    ao_pairs = []
                    for hp in range(HH // 2):
                        # -------- scores + exp (head pair, row-tiled PE) --------
                        pT = {}
                        for kt in range(nkt):
                            for hh in (0, 1):
                                sp = spsum.tile([128, 512], F32, tag="sp", name="sp")
                                nc.tensor.matmul(
                                    sp[:],
                                    lhsT=kt_sb[hp][hh * 64:(hh + 1) * 64,
                                                   kt * 128:(kt + 1) * 128],
                                    rhs=qt_sb[hp][hh * 64:(hh + 1) * 64,
                                                  qb * 512:(qb + 1) * 512],
                                    start=True, stop=True,
                                )
                                p = pT_pool.tile([128, 512], BF16, tag="p", name="p")
                                nc.scalar.activation(
                                    out=p[:], in_=sp[:],
                                    func=mybir.ActivationFunctionType.Exp,
                                    scale=float(SCALE))
                                if kt >= 4 * qb:
                                    # zero strict-upper (kpos > q) region of the
                                    # diagonal-crossing tile
                                    nc.gpsimd.affine_select(
                                        out=p[:], in_=p[:],
                                        compare_op=mybir.AluOpType.is_ge,
                                        fill=0.0,
                                        base=qb * 512 - kt * 128,
                                        channel_multiplier=-1,
                                        pattern=[[1, 512]])
                                pT[(kt, hh)] = p
                        # -------- U^T = [V|1]^T @ P^T (full 128x128 PE) --------
                        ao = aoT_pool.tile([128, 512], F32R, tag="aoT", name="aoT")
                        for hh in (0, 1):
                            h = 2 * hp + hh
                            u = upsum.tile([128, 512], F32, tag="u", name="u")
                            for kt in range(nkt):
                                nc.tensor.matmul(
                                    u[:],
                                    lhsT=v_sb[kt][:, h, :],
                                    rhs=pT[(kt, hh)][:],
                                    start=(kt == 0), stop=(kt == nkt - 1),
                                )
                            # rows 0-63: U^T; rows 64-127: denominator broadcast
                            rb = rb_pool.tile([128, 512], F32, tag="rb", name="rb")
                            nc.vector.reciprocal(rb[64:128, :], u[64:128, :])
                            nc.vector.tensor_mul(
                                out=ao[hh * 64:(hh + 1) * 64, :],
                                in0=u[0:64, :], in1=rb[64:128, :])
                        ao_pairs.append(ao)
                    # -------- out-projection for this q block --------
                    for qt_l in range(4):
                        qt = 4 * qb + qt_l
                        osb = out_pool.tile([128, D], F32, tag="osb", name="osb")
                        for half in range(2):
                            op = opsum.tile([128, 512], F32, tag="op", name="op")
                            for hp in range(HH // 2):
                                nc.tensor.matmul(
                                    op[:],
                                    lhsT=ao_pairs[hp][:, qt_l * 128:(qt_l + 1) * 128],
                                    rhs=wo_sb[hp][:, half * 512:(half + 1) * 512],
                                    start=(hp == 0), stop=(hp == 3),
                                )
                            nc.vector.tensor_copy(
                                out=osb[:, half * 512:(half + 1) * 512], in_=op[:])
                        nc.sync.dma_start(
                            out=out[qt * 128:(qt + 1) * 128, :], in_=osb[:])

    nc.compile()
    return nc



_NC_CACHE = {}


def _get_nc(s=S):
    if s not in _NC_CACHE:
        _NC_CACHE[s] = build_nc(s)
    return _NC_CACHE[s]


def make_in_maps(x, w_q, w_k, w_v, w_o, s=S):
    """Host-side sharding: returns the 8 per-core input maps."""
    x = np.ascontiguousarray(np.asarray(x, dtype=np.float32))
    w_q = np.asarray(w_q, dtype=np.float32)
    w_k = np.asarray(w_k, dtype=np.float32)
    w_v = np.asarray(w_v, dtype=np.float32)
    w_o = np.asarray(w_o, dtype=np.float32)

    xTs = [np.ascontiguousarray(x[b].T) for b in range(B)]
    wqTs = [np.ascontiguousarray(w_q[hg * HD:(hg + 1) * HD, :].T) for hg in range(2)]
    wkTs = [np.ascontiguousarray(w_k[hg * HD:(hg + 1) * HD, :].T) for hg in range(2)]
    wvTs = [np.ascontiguousarray(w_v[hg * HD:(hg + 1) * HD, :].T) for hg in range(2)]
    woTs = [np.ascontiguousarray(w_o[:, hg * HD:(hg + 1) * HD].T) for hg in range(2)]

    in_maps = []
    for c in range(N_CORES):
        b, hg = c // 2, c % 2
        in_maps.append({
            "xT": xTs[b], "wqT": wqTs[hg], "wkT": wkTs[hg],
            "wvT": wvTs[hg], "woT": woTs[hg],
        })
    return in_maps


def kernel(x, w_q, w_k, w_v, w_o, b_o):
    nc = _get_nc(S)
    in_maps = make_in_maps(x, w_q, w_k, w_v, w_o, s=S)
    res = run_bass_kernel_spmd(nc, in_maps, core_ids=list(range(N_CORES)))
    b_o = np.asarray(b_o, dtype=np.float32)
    outp = np.empty((B, S, D), dtype=np.float32)
    for b in range(B):
        outp[b] = res.results[2 * b]["out"] + res.results[2 * b + 1]["out"] + b_o
    return outp


# revision 7
# speedup vs baseline: 1.1290x; 1.1290x over previous
"""Multi-head causal attention (B=4, S=2048, D=1024, H=16) on 8 TRN2 NeuronCores.

Sharding: core c handles batch b = c//2 and head-group hg = c%2 (8 heads each).
Each core computes Q/K/V projections for its (batch, head-group), causal
attention, and a partial output projection over its 512 head-dims.  The host
sums the two partials per batch and adds b_o.  No collectives.

Device-side layout choices:
  - x is passed transposed (xT [D, S]) so projection matmuls contract over
    partitions directly.
  - Q and K are produced transposed (QT/KT [dq, S]); scores are computed
    transposed (S^T [kpos, q]) which makes the softmax denominator a matmul
    with a ones-column (no partition reductions anywhere).
  - No max-subtraction in softmax: scaled scores are ~N(0,1), exp is safe.
  - P (=exp(scores)) and V are bf16 for the P@V matmul; everything else is
    float32r (full-rate fp32 on the PE).
"""

import sys
import os

sys.path.insert(0, "/opt/trn_rl_repo")

import numpy as np

import concourse.bacc as bacc
import concourse.mybir as mybir
import concourse.tile as tile
from concourse.bass_utils import run_bass_kernel_spmd

B, S, D, H = 4, 2048, 1024, 16
DK = D // H          # 64
HH = H // 2          # 8 heads per core
HD = HH * DK         # 512 head-dims per core
N_CORES = 8

F32 = mybir.dt.float32
F32R = mybir.dt.float32r
BF16 = mybir.dt.bfloat16
F16 = mybir.dt.float16

SCALE = 1.0 / np.sqrt(DK)


def build_nc(s=S, interleave_pairs=True):
    """Build the per-core SPMD program.  `s` is the sequence length (tunable
    for small-scale simulation; must be a multiple of 512)."""
    assert s % 512 == 0
    n_qb = s // 512          # 512-wide q blocks
    n_t128 = s // 128        # 128-wide token tiles
    n_dt = D // 128          # din tiles (8)

    nc = bacc.Bacc("TRN2", target_bir_lowering=False, debug=False,
                   num_devices=N_CORES)

    xT = nc.dram_tensor("xT", [D, s], F16, kind="ExternalInput")
    wqT = nc.dram_tensor("wqT", [D, HD], F16, kind="ExternalInput")
    wkT = nc.dram_tensor("wkT", [D, HD], F16, kind="ExternalInput")
    wvT = nc.dram_tensor("wvT", [D, HD], F16, kind="ExternalInput")
    woT = nc.dram_tensor("woT", [HD, D], F16, kind="ExternalInput")
    out = nc.dram_tensor("out", [s, D], F32, kind="ExternalOutput")

    with tile.TileContext(nc) as tc:
        with tc.tile_pool(name="persist", bufs=1) as persist:
            # Persistent SBUF arrays (live for the whole kernel).
            qt_sb = [persist.tile([128, s], F16, tag=f"qt{d}", name=f"qt{d}") for d in range(HD // 128)]
            kt_sb = [persist.tile([128, s], F16, tag=f"kt{d}", name=f"kt{d}") for d in range(HD // 128)]
            # V tiles hold [t, head, 2*dk]: cols 0-63 are V, cols 64-127 are 1.0.
            # As the AV stationary this makes the matmul emit U^T on psum rows
            # 0-63 and the softmax denominator broadcast over rows 64-127.
            v_sb = [persist.tile([128, HH, 2 * DK], F16, tag=f"v{t}", name=f"v{t}") for t in range(n_t128)]
            wo_sb = [persist.tile([128, D], F16, tag=f"wo{d}", name=f"wo{d}") for d in range(HD // 128)]

            for d in range(HD // 128):
                nc.sync.dma_start(out=wo_sb[d][:], in_=woT[d * 128:(d + 1) * 128, :])

            # ---------------- Phase 1: projections ----------------
            with tc.tile_pool(name="xt", bufs=1) as xt_pool, \
                 tc.tile_pool(name="wload", bufs=10) as wload, \
                 tc.tile_pool(name="ppsum", bufs=4, space="PSUM") as ppsum:

                xt = []
                for i in range(n_dt):
                    t = xt_pool.tile([128, s], F16, tag=f"x{i}", name=f"x{i}")
                    nc.sync.dma_start(out=t[:], in_=xT[i * 128:(i + 1) * 128, :])
                    xt.append(t)

                # QT / KT: [dq, t] = sum_din wT[din, dq]^T xT[din, t]
                for (wdram, dst) in ((wqT, qt_sb), (wkT, kt_sb)):
                    w_tiles = []
                    for i in range(n_dt):
                        wt = wload.tile([128, HD], F16, tag="w", name="w")
                        nc.sync.dma_start(out=wt[:], in_=wdram[i * 128:(i + 1) * 128, :])
                        w_tiles.append(wt)
                    for dq in range(HD // 128):
                        for tb in range(n_qb):
                            ps = ppsum.tile([128, 512], F32, tag="pp", name="pp")
                            for i in range(n_dt):
                                nc.tensor.matmul(
                                    ps[:],
                                    lhsT=w_tiles[i][:, dq * 128:(dq + 1) * 128],
                                    rhs=xt[i][:, tb * 512:(tb + 1) * 512],
                                    start=(i == 0), stop=(i == n_dt - 1),
                                )
                            nc.vector.tensor_copy(
                                out=dst[dq][:, tb * 512:(tb + 1) * 512], in_=ps[:])

                # V: [t, dv] = sum_din xT[din, t]^T wvT[din, dv]
                wv_tiles = []
                for i in range(n_dt):
                    wt = wload.tile([128, HD], F16, tag="w", name="w")
                    nc.sync.dma_start(out=wt[:], in_=wvT[i * 128:(i + 1) * 128, :])
                    wv_tiles.append(wt)
                for tb in range(n_t128):
                    ps = ppsum.tile([128, 512], F32, tag="pp", name="pp")
                    for i in range(n_dt):
                        nc.tensor.matmul(
                            ps[:],
                            lhsT=xt[i][:, tb * 128:(tb + 1) * 128],
                            rhs=wv_tiles[i][:],
                            start=(i == 0), stop=(i == n_dt - 1),
                        )
                    # ones columns for the denominator, then V data (cast bf16)
                    nc.vector.memset(v_sb[tb][:, :, DK:2 * DK], 1.0)
                    nc.vector.tensor_copy(
                        out=v_sb[tb][:, :, 0:DK],
                        in_=ps[:].rearrange("p (h k) -> p h k", h=HH))

            # ---------------- Phase 2: attention + out-projection ----------------
            with tc.tile_pool(name="pT", bufs=40) as pT_pool, \
                 tc.tile_pool(name="aoT", bufs=6) as aoT_pool, \
                 tc.tile_pool(name="rb", bufs=4) as rb_pool, \
                 tc.tile_pool(name="outsb", bufs=3) as out_pool, \
                 tc.tile_pool(name="spsum", bufs=3, space="PSUM") as spsum, \
                 tc.tile_pool(name="upsum", bufs=3, space="PSUM") as upsum, \
                 tc.tile_pool(name="opsum", bufs=2, space="PSUM") as opsum:

                for qb in range(n_qb):
                    nkt = 4 * qb + 4
                    ao_pairs = []
                    for hp in range(HH // 2):
                        # -------- scores + exp (head pair, row-tiled PE) --------
                        pT = {}
                        for kt in range(nkt):
                            for hh in (0, 1):
                                sp = spsum.tile([128, 512], F32, tag="sp", name="sp")
                                nc.tensor.matmul(
                                    sp[:],
                                    lhsT=kt_sb[hp][hh * 64:(hh + 1) * 64,
                                                   kt * 128:(kt + 1) * 128],
                                    rhs=qt_sb[hp][hh * 64:(hh + 1) * 64,
                                                  qb * 512:(qb + 1) * 512],
                                    start=True, stop=True,
                                )
                                p = pT_pool.tile([128, 512], F16, tag="p", name="p")
                                nc.scalar.activation(
                                    out=p[:], in_=sp[:],
                                    func=mybir.ActivationFunctionType.Exp,
                                    scale=float(SCALE))
                                if kt >= 4 * qb:
                                    # zero strict-upper (kpos > q) region of the
                                    # diagonal-crossing tile
                                    nc.gpsimd.affine_select(
                                        out=p[:], in_=p[:],
                                        compare_op=mybir.AluOpType.is_ge,
                                        fill=0.0,
                                        base=qb * 512 - kt * 128,
                                        channel_multiplier=-1,
                                        pattern=[[1, 512]])
                                pT[(kt, hh)] = p
                        # -------- U^T = [V|1]^T @ P^T (full 128x128 PE) --------
                        ao = aoT_pool.tile([128, 512], F16, tag="aoT", name="aoT")
                        for hh in (0, 1):
                            h = 2 * hp + hh
                            u = upsum.tile([128, 512], F32, tag="u", name="u")
                            for kt in range(nkt):
                                nc.tensor.matmul(
                                    u[:],
                                    lhsT=v_sb[kt][:, h, :],
                                    rhs=pT[(kt, hh)][:],
                                    start=(kt == 0), stop=(kt == nkt - 1),
                                )
                            # rows 0-63: U^T; rows 64-127: denominator broadcast
                            rb = rb_pool.tile([128, 512], F32, tag="rb", name="rb")
                            nc.vector.reciprocal(rb[64:128, :], u[64:128, :])
                            nc.vector.tensor_mul(
                                out=ao[hh * 64:(hh + 1) * 64, :],
                                in0=u[0:64, :], in1=rb[64:128, :])
                        ao_pairs.append(ao)
                    # -------- out-projection for this q block --------
                    for qt_l in range(4):
                        qt = 4 * qb + qt_l
                        osb = out_pool.tile([128, D], F32, tag="osb", name="osb")
                        for half in range(2):
                            op = opsum.tile([128, 512], F32, tag="op", name="op")
                            for hp in range(HH // 2):
                                nc.tensor.matmul(
                                    op[:],
                                    lhsT=ao_pairs[hp][:, qt_l * 128:(qt_l + 1) * 128],
                                    rhs=wo_sb[hp][:, half * 512:(half + 1) * 512],
                                    start=(hp == 0), stop=(hp == 3),
                                )
                            nc.vector.tensor_copy(
                                out=osb[:, half * 512:(half + 1) * 512], in_=op[:])
                        nc.sync.dma_start(
                            out=out[qt * 128:(qt + 1) * 128, :], in_=osb[:])

    nc.compile()
    return nc



_NC_CACHE = {}


def _get_nc(s=S):
    if s not in _NC_CACHE:
        _NC_CACHE[s] = build_nc(s)
    return _NC_CACHE[s]


def make_in_maps(x, w_q, w_k, w_v, w_o, s=S):
    """Host-side sharding: returns the 8 per-core input maps."""
    x = np.ascontiguousarray(np.asarray(x, dtype=np.float32))
    w_q = np.asarray(w_q, dtype=np.float32)
    w_k = np.asarray(w_k, dtype=np.float32)
    w_v = np.asarray(w_v, dtype=np.float32)
    w_o = np.asarray(w_o, dtype=np.float32)

    xTs = [np.ascontiguousarray(x[b].T.astype(np.float16)) for b in range(B)]
    wqTs = [np.ascontiguousarray(w_q[hg * HD:(hg + 1) * HD, :].T.astype(np.float16)) for hg in range(2)]
    wkTs = [np.ascontiguousarray(w_k[hg * HD:(hg + 1) * HD, :].T.astype(np.float16)) for hg in range(2)]
    wvTs = [np.ascontiguousarray(w_v[hg * HD:(hg + 1) * HD, :].T.astype(np.float16)) for hg in range(2)]
    woTs = [np.ascontiguousarray(w_o[:, hg * HD:(hg + 1) * HD].T.astype(np.float16)) for hg in range(2)]

    in_maps = []
    for c in range(N_CORES):
        b, hg = c // 2, c % 2
        in_maps.append({
            "xT": xTs[b], "wqT": wqTs[hg], "wkT": wkTs[hg],
            "wvT": wvTs[hg], "woT": woTs[hg],
        })
    return in_maps


def kernel(x, w_q, w_k, w_v, w_o, b_o):
    nc = _get_nc(S)
    in_maps = make_in_maps(x, w_q, w_k, w_v, w_o, s=S)
    res = run_bass_kernel_spmd(nc, in_maps, core_ids=list(range(N_CORES)))
    b_o = np.asarray(b_o, dtype=np.float32)
    outp = np.empty((B, S, D), dtype=np.float32)
    for b in range(B):
        outp[b] = res.results[2 * b]["out"] + res.results[2 * b + 1]["out"] + b_o
    return outp
